# revision 29
# baseline (speedup 1.0000x reference)
"""Trainium2 Bass kernel for nn_AttentionSheafLearner.

Computation:  maps = x[row] @ W[:, :C].T + x[col] @ W[:, C:].T    [E, 25]
              out  = eye(5) - softmax(maps.reshape(E, 5, 5), axis=-1)

Strategy (8 NeuronCores, SPMD):
  - Precompute z[n] = [x[n] @ Wr.T | x[n] @ Wc.T | pad]  (64 f32 = 256B rows)
    on device with bf16 PE matmuls; per node-half tables in DRAM.  Table rows
    are renumbered so node n lives at slot (n%128)*NCH + n//128, which makes
    the stage-A z stores per-partition contiguous (large HWDGE descriptors).
  - Edges sharded by VALUE class: nodes split in two halves (A = <25088),
    edge class = (row_half, col_half); each class handled by 2 cores so
    per-core gather indices fit int16 (SWDGE dma_gather idx dtype).
  - Row-side gather dedup: edges are grouped by row node into multiplicity
    classes (class k = groups of k edges sharing one row). The row z is
    gathered ONCE per group and expanded on-chip with a stride-0 broadcast
    AP in the DVE add; only the col side is gathered per edge.
    (Known dead ends, measured: single_packet=True crashes above 64
    descs/engine and wins nothing below; trailing -1 idx pads crash the
    device; SDMA drain and DVE are not the bottleneck.)
  - All gather indices are preloaded in ONE DMA into a queue-banded SBUF
    tile (queue q's Q7 pair reads partitions [32q,32q+32)), killing the
    per-chunk idx loads and Pool-engine semaphore waits.
  - Gathers are split into ~1-1.5k-idx pieces round-robined over the 4
    SWDGE queues; each queue's descriptor generation runs on a distinct
    Q7 core pair (queue_num == cpu_id/2 in the ucode), so desc-gen (the
    bottleneck, ~8.8ns/idx per pair) runs ~3x parallel.  Emission order:
    col pieces of the first FC chunks (needing only the z_c table, built
    first), then every chunk's row piece (z_r-dependent, resident for the
    whole stage), then the remaining col pieces -- so desc-gen starts as
    soon as z_c lands instead of after all of stage A.
  - Explicit add_dep_helper RAW edges gather<-last-z-store (the sync-ring
    HWDGE stores complete FIFO) -- the automatic DRAM dep tracking misses
    the rearranged gather in_ap and the gathers race the table writes.
  - Per chunk: DVE add -> maps, ScalarE exp, DVE strided reduce /
    reciprocal_approx_fast / broadcast-mul, out = eye - sm, store.
  - Host re-permutes the output rows back to original edge order.

The class layout (quotas per multiplicity class) is computed from the actual
edge_index at kernel() time; the program is built and compiled per call.
"""

import os

import numpy as np

# problem sizes (hardcoded per contract)
N = 50000
C = 128
D = 5
DD = D * D          # 25
E = 1_600_000
NCORES = 8
P = 128

HALF = 25088        # nodes per half (padded; 2*HALF >= N)
ZW = 64             # z row width in f32 (256B, dma_gather elem size)
NCH_H = HALF // P   # 196 node chunks per half
KMAX = 14           # max row-multiplicity class; larger rows are decomposed
COL_TARGET = 40     # target col-slot columns per chunk
SU_CAP = 20         # cap on row-groups per chunk (bounds g_r tile size)
NSPLIT = 4          # col gather pieces per chunk (spread over queues)
NQ = 4              # SWDGE queues (= Q7 core pairs)
FC = 5              # chunks whose col pieces are emitted before the rows

_XBLK = 14          # node chunks per xT DMA block
_ZGRP = 7           # node chunks per z store group


def _su_for(kk):
    return min(SU_CAP, max(1, COL_TARGET // kk))


def _tidx(n):
    """Renumbered z-table slot for local node id n (vectorized)."""
    return (n % P) * NCH_H + n // P


class _Layout:
    """Per-run static layout shared by host prep and program build.

    quotas[k] = padded group count (multiple of 128) for class k, the max
    over cores. chunks: list of (kk, su, u0, q, r_off, c_off, o_off) with
    u0 in group-columns (128 groups per column), q the SWDGE queue,
    r_off/c_off idx-tile int16 column offsets (16-wrapped) WITHIN queue q's
    band, o_off in output slot-columns.
    """

    def __init__(self, quotas):
        self.quotas = dict(quotas)           # kk -> padded group count
        self.chunks = []                     # (kk, su, u0, o_off)
        o_off = 0
        for kk in sorted(self.quotas, reverse=True):
            qq = self.quotas[kk]
            assert qq % P == 0
            su_full = _su_for(kk)
            ncols = qq // P
            u0 = 0
            while u0 < ncols:
                su = min(su_full, ncols - u0)
                self.chunks.append((kk, su, u0, o_off))
                o_off += su * kk
                u0 += su
        self.t_cols = o_off                  # total output slot-columns

        # emission order: col pieces of chunks [0,FC), then all row pieces,
        # then col pieces of chunks [FC,N).  Queues round-robin in emission
        # order so adjacent Pool instructions hit different Q7 pairs.
        # pieces[ci] = {'r': (q, off, n), 'c': [(q, off, n, lo), ...]}
        self.pieces = [dict(c=[]) for _ in self.chunks]
        self.emission = []                   # ('r'|'c', ci, piece_idx)
        q_off = [0] * NQ
        gi = 0

        def _alloc(n):
            nonlocal gi
            q = gi % NQ
            gi += 1
            off = q_off[q]
            q_off[q] += n // 16
            return q, off

        def _emit_cols(ci):
            kk, su, u0, o_off_c = self.chunks[ci]
            X = su * kk
            bounds = [X * j // NSPLIT for j in range(NSPLIT)] + [X]
            for j in range(NSPLIT):
                lo, hi = bounds[j], bounds[j + 1]
                if hi <= lo:
                    continue
                n = P * (hi - lo)
                q, off = _alloc(n)
                self.pieces[ci]["c"].append((q, off, n, lo))
                self.emission.append(("c", ci, len(self.pieces[ci]["c"]) - 1))

        def _emit_row(ci):
            kk, su, u0, o_off_c = self.chunks[ci]
            n = P * su
            q, off = _alloc(n)
            self.pieces[ci]["r"] = (q, off, n)
            self.emission.append(("r", ci, 0))

        nch = len(self.chunks)
        for ci in range(min(FC, nch)):
            _emit_cols(ci)
        for ci in range(nch):
            _emit_row(ci)
        for ci in range(min(FC, nch), nch):
            _emit_cols(ci)
        self.idx_cols = max(q_off)


def _build_layout(row_locals):
    """row_locals: list per core of int arrays (local row ids). Returns
    (_Layout, groups_per_core) where groups_per_core[core][kk] is a list of
    (row_id, edge_positions array of length kk).

    Quota equalization: classes k>=2 get quotas near the per-core median
    (rounded up to 128); cores above quota split excess k-groups into a
    (k-1)-group + a 1-group (cascading downward).  A few hundred extra
    groups replace ~10k max-padding idx per core."""
    per_core = []
    counts = []
    for rl in row_locals:
        order = np.argsort(rl, kind="stable")
        sr = rl[order]
        starts = np.flatnonzero(np.r_[True, sr[1:] != sr[:-1]])
        ends = np.r_[starts[1:], len(sr)]
        groups = {kk: [] for kk in range(1, KMAX + 1)}
        for s, e in zip(starts, ends):
            r = int(sr[s])
            pos = order[s:e]
            o = 0
            k = e - s
            while k > 0:
                kk = min(k, KMAX)
                groups[kk].append((r, pos[o:o + kk]))
                o += kk
                k -= kk
        per_core.append(groups)
        counts.append({kk: len(g) for kk, g in groups.items()})

    quotas = {}
    for kk in range(KMAX, 1, -1):
        med = int(np.median([c[kk] for c in counts]))
        quotas[kk] = -(-max(med, 1) // P) * P
        for groups in per_core:
            g = groups[kk]
            while len(g) > quotas[kk]:
                r, pos = g.pop()
                groups[kk - 1].append((r, pos[:kk - 1]))
                groups[1].append((r, pos[kk - 1:]))
            # re-check cascaded growth of class kk-1 on its own turn
    q1 = max(len(groups[1]) for groups in per_core)
    quotas[1] = -(-max(q1, 1) // P) * P
    per_core = [
        {kk: g for kk, g in groups.items() if quotas.get(kk, 0) > 0}
        for groups in per_core
    ]
    quotas = {kk: q for kk, q in quotas.items() if q > 0}
    return _Layout(quotas), per_core


def _build_nc(layout):
    from contextlib import ExitStack

    import concourse.bacc as bacc
    import concourse.mybir as mybir
    import concourse.tile as tile

    f32 = mybir.dt.float32
    bf16 = mybir.dt.bfloat16
    i16 = mybir.dt.int16

    nc = bacc.Bacc(
        "TRN2",
        target_bir_lowering=False,
        debug=False,
        enable_asserts=False,
        num_devices=NCORES,
        num_swdge_queues=NQ,
    )

    xt_r_d = nc.dram_tensor("xt_r", [P, HALF], bf16, kind="ExternalInput")
    xt_c_d = nc.dram_tensor("xt_c", [P, HALF], bf16, kind="ExternalInput")
    w_d = nc.dram_tensor("w", [P, 2 * DD], bf16, kind="ExternalInput")
    idx_d = nc.dram_tensor("idx", [P, layout.idx_cols], i16, kind="ExternalInput")
    eye_d = nc.dram_tensor("eye", [P, DD], f32, kind="ExternalInput")
    z_r_d = nc.dram_tensor("z_r", [P, NCH_H * ZW], f32)
    z_c_d = nc.dram_tensor("z_c", [P, NCH_H * ZW], f32)
    # dummy warm-up table sized for the full idx range (warm-up gathers use
    # real idx values; keep their reads in-bounds)
    wtab_d = nc.dram_tensor("wtab", [HALF, ZW], f32)
    out_d = nc.dram_tensor("out", [P * layout.t_cols, DD], f32, kind="ExternalOutput")

    oview = out_d.ap().rearrange("(p t) d -> p (t d)", p=P)  # [128, t_cols*25]
    zr_rows = z_r_d.ap().rearrange("p (i d) -> (p i) d", d=ZW)
    zc_rows = z_c_d.ap().rearrange("p (i d) -> (p i) d", d=ZW)

    with tile.TileContext(nc) as tc, ExitStack() as ctx:
        const_pool = ctx.enter_context(tc.tile_pool(name="const", bufs=1))
        # all gather indices, queue-banded; one DMA
        i_tile = const_pool.tile([P, layout.idx_cols], i16)
        nc.sync.dma_start(i_tile[:], idx_d.ap())
        w_tile = const_pool.tile([P, 2 * DD], bf16)
        nc.sync.dma_start(w_tile[:], w_d.ap())
        eye_tile = const_pool.tile([P, DD], f32)
        nc.sync.dma_start(eye_tile[:], eye_d.ap())

        # ---- stage A: z tables (bf16 matmuls, f32 tables, dense stores) ----
        z_last = {}          # "z_c"/"z_r" -> last store BassInstruction
        with ExitStack() as actx:
            xt_pool = actx.enter_context(tc.tile_pool(name="xt", bufs=6))
            z_pool = actx.enter_context(tc.tile_pool(name="zsb", bufs=4))
            ps_pool = actx.enter_context(
                tc.tile_pool(name="ps", bufs=6, space="PSUM")
            )
            for zname, xt_d, z_d in (
                ("z_c", xt_c_d, z_c_d),
                ("z_r", xt_r_d, z_r_d),
            ):
                xt_tiles = []
                for blk in range(NCH_H // _XBLK):  # 14
                    xt_tile = xt_pool.tile([P, _XBLK * P], bf16)
                    nc.sync.dma_start(
                        xt_tile[:],
                        xt_d.ap()[:, blk * _XBLK * P:(blk + 1) * _XBLK * P],
                    )
                    xt_tiles.append(xt_tile)
                for blk in range(NCH_H // _XBLK):  # 14
                    xt_tile = xt_tiles[blk]
                    for grp in range(_XBLK // _ZGRP):  # 2
                        z_sb = z_pool.tile([P, _ZGRP * ZW], f32)
                        ps = ps_pool.tile([P, _ZGRP * 2 * DD], f32, space="PSUM")
                        for j in range(_ZGRP):
                            jj = grp * _ZGRP + j
                            nc.tensor.matmul(
                                ps[:, j * 2 * DD:(j + 1) * 2 * DD],
                                xt_tile[:, jj * P:(jj + 1) * P],
                                w_tile[:],
                                start=True,
                                stop=True,
                            )
                        nc.vector.tensor_copy(
                            z_sb[:]
                            .rearrange("p (i d) -> p i d", i=_ZGRP)[:, :, 0:2 * DD],
                            ps[:].rearrange("p (i d) -> p i d", i=_ZGRP),
                        )
                        i0 = blk * _XBLK + grp * _ZGRP
                        z_last[zname] = nc.sync.dma_start(
                            z_d.ap()[:, i0 * ZW:(i0 + _ZGRP) * ZW], z_sb[:]
                        )

        # ---- stage B: gather + softmax per class chunk ----
        # per-class row pools with one buffer per chunk: every chunk's g_r
        # stays resident for the whole stage (rows are gathered up front,
        # overlapping the z_r build)
        from collections import Counter

        cls_count = Counter(kk for (kk, su, u0, o_off) in layout.chunks)
        gr_pools = {
            kk: ctx.enter_context(
                tc.tile_pool(name=f"gr{kk}", bufs=cnt)
            )
            for kk, cnt in cls_count.items()
        }
        gc_pool = ctx.enter_context(tc.tile_pool(name="gc", bufs=FC + 2))
        m_pool = ctx.enter_context(tc.tile_pool(name="m", bufs=3))
        e_pool = ctx.enter_context(tc.tile_pool(name="e", bufs=3))
        s_pool = ctx.enter_context(tc.tile_pool(name="s", bufs=3))
        o_pool = ctx.enter_context(tc.tile_pool(name="o", bufs=3))
        wu_pool = ctx.enter_context(tc.tile_pool(name="wu", bufs=1))

        # warm-up: tiny gathers on each queue load the Q7 ext-isa IRAM and
        # init the rings at t=0 (dummy table: no dependency on z)
        wu = wu_pool.tile([P, ZW], f32)
        for q in range(NQ):
            nc.gpsimd.dma_gather(
                out_ap=wu[:].rearrange("p (u d) -> p u d", d=ZW),
                in_ap=wtab_d.ap(),
                idxs_ap=i_tile[:, 0:8],
                num_idxs=P,
                num_idxs_reg=P,
                elem_size=ZW,
                single_packet=False,
                queue_num=q,
            )

        g_r_t = {}
        g_c_t = {}
        done = Counter()

        def emit_chain(ci):
            kk, su, u0, o_off = layout.chunks[ci]
            X = su * kk
            g_r, g_c = g_r_t[ci], g_c_t[ci]
            m = m_pool.tile([P, X * DD], f32)
            nc.vector.tensor_tensor(
                out=m[:].rearrange("p (u m d) -> p u m d", u=su, m=kk),
                in0=g_r[:]
                .rearrange("p (u d) -> p u d", d=ZW)[:, :, 0:DD]
                .unsqueeze(2)
                .to_broadcast([P, su, kk, DD]),
                in1=g_c[:].rearrange("p (u m d) -> p u m d", u=su, d=ZW)[
                    :, :, :, DD:2 * DD
                ],
                op=mybir.AluOpType.add,
            )
            et = e_pool.tile([P, X * DD], f32)
            nc.scalar.activation(et[:], m[:], mybir.ActivationFunctionType.Exp)
            e3 = et[:].rearrange("p (t d) -> p t d", d=D)  # [128, X*5, 5]
            s = s_pool.tile([P, X * D], f32, tag="s")
            nc.vector.reduce_sum(s[:], e3, axis=mybir.AxisListType.X)
            r = s_pool.tile([P, X * D], f32, tag="r")
            nc.vector.reciprocal_approx_fast(out=r[:], in_=s[:])
            o = o_pool.tile([P, X * DD], f32)
            nc.vector.tensor_tensor(
                out=o[:].rearrange("p (t d) -> p t d", d=D),
                in0=e3,
                in1=r[:].unsqueeze(2).to_broadcast([P, X * D, D]),
                op=mybir.AluOpType.mult,
            )
            nc.vector.tensor_tensor(
                out=o[:].rearrange("p (t d) -> p t d", d=DD),
                in0=eye_tile[:].unsqueeze(1).to_broadcast([P, X, DD]),
                in1=o[:].rearrange("p (t d) -> p t d", d=DD),
                op=mybir.AluOpType.subtract,
            )
            nc.sync.dma_start(
                oview[:, o_off * DD:(o_off + X) * DD], o[:]
            )

        for (kind, ci, pj) in layout.emission:
            kk, su, u0, o_off = layout.chunks[ci]
            X = su * kk
            if kind == "r":
                q, off, n = layout.pieces[ci]["r"]
                g_r = gr_pools[kk].tile([P, su * ZW], f32, tag="gr")
                g_r_t[ci] = g_r
                gi = nc.gpsimd.dma_gather(
                    out_ap=g_r[:].rearrange("p (u d) -> p u d", d=ZW),
                    in_ap=zr_rows,
                    idxs_ap=i_tile[:, off:off + n // 16],
                    num_idxs=n,
                    num_idxs_reg=n,
                    elem_size=ZW,
                    single_packet=False,
                    queue_num=q,
                )
                # explicit RAW: z_r stores (FIFO on the sync HWDGE ring, so
                # the last one completing implies all) -> this gather
                tile.add_dep_helper(
                    gi.ins, z_last["z_r"].ins, reason="z_r table RAW"
                )
            else:
                q, off, n, lo = layout.pieces[ci]["c"][pj]
                if ci not in g_c_t:
                    g_c_t[ci] = gc_pool.tile(
                        [P, X * ZW], f32, tag="gc", name=f"gc{ci}"
                    )
                gcv = g_c_t[ci][:].rearrange("p (u d) -> p u d", d=ZW)
                gi = nc.gpsimd.dma_gather(
                    out_ap=gcv[:, lo:lo + n // P],
                    in_ap=zc_rows,
                    idxs_ap=i_tile[:, off:off + n // 16],
                    num_idxs=n,
                    num_idxs_reg=n,
                    elem_size=ZW,
                    single_packet=False,
                    queue_num=q,
                )
                tile.add_dep_helper(
                    gi.ins, z_last["z_c"].ins, reason="z_c table RAW"
                )
            done[ci] += 1
            if done[ci] == 1 + len(layout.pieces[ci]["c"]):
                emit_chain(ci)

    nc.compile()
    return nc


def _wrap16(a):
    """Gather-order idx list [n] -> [16, n//16] int16 (16-wrapped)."""
    n = len(a)
    assert n % 16 == 0
    return np.ascontiguousarray(a.reshape(n // 16, 16).T.astype(np.int16))


def _host_prep(x, W, edge_index):
    """Shard edges by (row_half, col_half) class across cores; group by row
    node into multiplicity classes; build per-core inputs. Returns
    (layout, in_maps, slot_maps) where slot_maps[c] = (edge_ids, out_rows)."""
    x = np.asarray(x, dtype=np.float32)
    W = np.asarray(W, dtype=np.float32)
    ei = np.asarray(edge_index)
    row = ei[0].astype(np.int64)
    col = ei[1].astype(np.int64)

    import ml_dtypes

    bf16 = ml_dtypes.bfloat16
    xt = np.zeros((P, 2 * HALF), dtype=np.float32)
    xt[:, :N] = x.T
    xt_half = [
        np.ascontiguousarray(xt[:, :HALF].astype(bf16)),
        np.ascontiguousarray(xt[:, HALF:].astype(bf16)),
    ]

    w = np.zeros((P, 2 * DD), dtype=np.float32)
    w[:, :DD] = W[:, :C].T
    w[:, DD:2 * DD] = W[:, C:].T
    w = w.astype(bf16)
    eye = np.ascontiguousarray(
        np.broadcast_to(np.eye(D, dtype=np.float32).reshape(1, DD), (P, DD))
    )

    cls = (row >= HALF).astype(np.int64) * 2 + (col >= HALF)
    order = np.argsort(cls, kind="stable")
    counts = np.bincount(cls, minlength=4)
    starts = np.concatenate([[0], np.cumsum(counts)])

    subs = []
    row_locals = []
    for core in range(NCORES):
        k = core // 2
        sub = order[starts[k]:starts[k + 1]][core % 2::2]
        subs.append(sub)
        row_locals.append((row[sub] - (k >> 1) * HALF).astype(np.int64))

    layout, per_core_groups = _build_layout(row_locals)

    in_maps = []
    slot_maps = []
    for core in range(NCORES):
        k = core // 2
        half_r, half_c = k >> 1, k & 1
        sub = subs[core]
        col_local = (col[sub] - half_c * HALF).astype(np.int64)
        groups = per_core_groups[core]

        # per class: padded group arrays (row id + kk edge positions);
        # pads marked -1 (turned into trailing -1 idx or node-0 gathers by
        # _place; never a fully-empty idx stream, which crashes the device)
        cls_rows = {}
        cls_edges = {}
        for kk, q in layout.quotas.items():
            g = groups.get(kk, [])
            rows_a = np.full(q, -1, dtype=np.int64)
            edges_a = np.full((q, kk), -1, dtype=np.int64)
            for i, (r, pos) in enumerate(g):
                rows_a[i] = r
                edges_a[i] = pos
            cls_rows[kk] = rows_a
            cls_edges[kk] = edges_a

        # build the queue-banded idx mega-array + per chunk slot maps
        idx_arr = np.zeros((P, layout.idx_cols), dtype=np.int16)
        eids = []
        orows = []
        t_cols = layout.t_cols
        def _place(q, off, vals, pad_mask):
            # trailing pads -> -1 (SWDGE skips trailing negatives), but keep
            # idx[0] valid: a num_idxs=0 gather crashes the device
            vals = vals.copy()
            if False and pad_mask is not None and pad_mask.any():
                valid_pos = np.flatnonzero(~pad_mask)
                last = valid_pos[-1] + 1 if len(valid_pos) else 1
                vals[last:] = -1
            wv = _wrap16(vals)
            for rep in range(2):
                p0 = 32 * q + 16 * rep
                idx_arr[p0:p0 + 16, off:off + wv.shape[1]] = wv

        for ci, (kk, su, u0, o_off) in enumerate(layout.chunks):
            g0 = u0 * P
            rows_a = cls_rows[kk][g0:g0 + su * P]          # [su*128]
            edges_a = cls_edges[kk][g0:g0 + su * P]        # [su*128, kk]
            # row idx: table-renumbered; pads gather node 0 (harmless)
            rv = _tidx(np.maximum(rows_a, 0))
            rpad = rows_a < 0
            # col idx order: u-major, pos = (u*kk + m)*128 + p
            e_um = edges_a.reshape(su, P, kk).transpose(0, 2, 1)  # [su, kk, P]
            cv = np.where(
                e_um >= 0, _tidx(col_local[np.maximum(e_um, 0)]), 0
            ).reshape(-1)
            cpad = (e_um < 0).reshape(-1)
            # place wrapped idx into each piece's queue band (x2 replication)
            q, off, n = layout.pieces[ci]["r"]
            _place(q, off, rv, rpad)
            for (q, off, n, lo) in layout.pieces[ci]["c"]:
                _place(q, off, cv[lo * P:lo * P + n], cpad[lo * P:lo * P + n])
            # out slot rows: edge (u, m, p) -> out row p*t_cols + (o_off + u*kk + m)
            uu, mm, pp = np.meshgrid(
                np.arange(su), np.arange(kk), np.arange(P), indexing="ij"
            )
            valid = e_um >= 0
            rws = pp * t_cols + (o_off + uu * kk + mm)
            eids.append(sub[e_um[valid]])
            orows.append(rws[valid])
        in_maps.append(
            {
                "xt_r": xt_half[half_r],
                "xt_c": xt_half[half_c],
                "w": w,
                "eye": eye,
                "idx": idx_arr,
            }
        )
        slot_maps.append(
            (np.concatenate(eids), np.concatenate(orows))
        )
    return layout, in_maps, slot_maps


LAST_EXEC_NS = None


def kernel(x, W, edge_index):
    global LAST_EXEC_NS
    from concourse.bass_utils import run_bass_kernel_spmd

    layout, in_maps, slot_maps = _host_prep(x, W, edge_index)
    nc = _build_nc(layout)
    trace = os.environ.get("KERNEL_TRACE", "0") == "1"
    br = run_bass_kernel_spmd(
        nc,
        in_maps,
        core_ids=list(range(NCORES)),
        trace=trace,
    )
    LAST_EXEC_NS = br.exec_time_ns

    out = np.empty((E, DD), dtype=np.float32)
    for core in range(NCORES):
        res = br.results[core]["out"]        # [P*t_cols, 25]
        eids, orows = slot_maps[core]
        out[eids] = res[orows]
    return out.reshape(E, D, D).astype(np.float32)
